# revision 12
# baseline (speedup 1.0000x reference)
"""Trainium2 Bass kernel for nn_DirectEncodingModel (gnn_message_passing).

Strategy
--------
The per-level gather + grouped einsum is linear in the activations, so on the
host we fold gather+weights into one dense matrix per level:
    out_l = tanh(flat @ W_l + b_l),   W_l[c, g*U+u] = sum_{f: idx_l[g,f]==c} K_l[g,f,u]
with flat = concat(x, out_1, ..., out_{l-1}) along features.

On-chip we keep activations feature-major ([feature, batch]) so each level is a
chain of [K=128, M=128] x [K=128, N] matmuls accumulating in PSUM, then one
ACT-engine tanh(+bias) per 128-feature chunk straight out of PSUM. All
intermediate state stays in SBUF; HBM traffic is x in + out4 out only.

Numerics: weights and activations are fp16 on the matmul path (10-bit mantissa,
same rounding as TF32 for normal-range values; accumulation is fp32 in PSUM).
The final level's tanh is written in fp32. End-to-end error ~1.9e-3 absmax,
dominated by the fp16/TF32 operand rounding. (A float32r variant with
identical accuracy exists behind mode="f32r"; fp16 is faster because the
2-byte weight load pipelines and the moving operand can be 1024 wide.)

Sharding: pure data parallelism - batch split across 8 NeuronCores, weights
replicated, each core handles 8192 rows.
"""

import numpy as np

B = 65536
N_IN = 256
G = 16
U = 16
F = 32
LEVELS = 4
NCORES = 8
BS = B // NCORES          # 8192 rows per core
KCH = [2, 4, 6, 8]        # K-chunks (128 feats) per level: C_l/128
NWCOLS = sum(KCH) * 2 * 128  # 5120 weight columns

MODE = "f16"              # "f16" or "f32r"


def _round_tf32(a):
    u = np.ascontiguousarray(a, np.float32).view(np.uint32)
    u = ((u.astype(np.uint64) + 0x1000) & 0xFFFFE000).astype(np.uint32)
    return u.view(np.float32)


def _build_nc(hw_loop=0, mode=MODE):
    from concourse import bacc, mybir
    import concourse.tile as tile

    F32 = mybir.dt.float32
    Tanh = mybir.ActivationFunctionType.Tanh
    if mode == "f16":
        ADT = WDT = mybir.dt.float16
    else:
        ADT = WDT = mybir.dt.float32r
    NT = 512               # moving-operand free size is ISA-capped at 512
    CHUNK = 2048           # batch columns per chunk (= ACT batch width)
    TPC = CHUNK // NT

    nc = bacc.Bacc("TRN2", target_bir_lowering=False, debug=False)
    wpack_d = nc.dram_tensor("wpack", [128, NWCOLS], WDT, kind="ExternalInput").ap()
    bpack_d = nc.dram_tensor("bpack", [128, 2 * LEVELS], F32, kind="ExternalInput").ap()
    xT_d = nc.dram_tensor("xT", [256, BS], ADT, kind="ExternalInput").ap()
    outT_d = nc.dram_tensor("outT", [256, BS], F32, kind="ExternalOutput").ap()

    with tile.TileContext(nc) as tc:
        with (
            tc.tile_pool(name="wpool", bufs=1) as wpool,
            tc.tile_pool(name="xpool", bufs=3) as xpool,
            tc.tile_pool(name="stgpool", bufs=4) as stgpool,
            tc.tile_pool(name="actpool", bufs=14) as actpool,
            tc.tile_pool(name="opool", bufs=3) as opool,
            tc.tile_pool(name="psum", bufs=8, space="PSUM") as psum_pool,
        ):
            wp = wpool.tile([128, NWCOLS], WDT)
            nc.sync.dma_start(wp[:], wpack_d[:])
            bp = wpool.tile([128, 2 * LEVELS], F32)
            nc.sync.dma_start(bp[:], bpack_d[:])

            # weight chunk APs: (level, kchunk, mchunk) -> [128, 128]
            Wc = {}
            i = 0
            for l in range(LEVELS):
                for k in range(KCH[l]):
                    for m in range(2):
                        Wc[(l, k, m)] = wp[:, i * 128:(i + 1) * 128]
                        i += 1
            bias = {(l, m): bp[:, l * 2 + m:l * 2 + m + 1]
                    for l in range(LEVELS) for m in range(2)}

            def start_chunk(ch):
                c0 = (ch % (BS // CHUNK)) * CHUNK
                xa = xpool.tile([128, CHUNK], ADT, tag="x0", name="xa")
                xb = xpool.tile([128, CHUNK], ADT, tag="x1", name="xb")
                nc.sync.dma_start(xa[:], xT_d[0:128, c0:c0 + CHUNK])
                nc.sync.dma_start(xb[:], xT_d[128:256, c0:c0 + CHUNK])
                # acts[tt] = list of [128, NT] feature-chunk APs of `flat`
                acts = [
                    [xa[:, tt * NT:(tt + 1) * NT], xb[:, tt * NT:(tt + 1) * NT]]
                    for tt in range(TPC)
                ]
                return {"c0": c0, "acts": acts}

            def emit_level(st, l):
                # PE fills PSUM banks; DVE drains them into a [128, CHUNK] fp32
                # staging tile (fast bank release); ACT does ONE wide tanh.
                nk = KCH[l]
                for m in range(2):
                    stg = stgpool.tile([128, CHUNK], F32, tag="stg", name="stg")
                    for tt in range(TPC):
                        ps = psum_pool.tile([128, NT], F32, tag="ps", name="ps")
                        rhs = st["acts"][tt]
                        for k in range(nk):
                            nc.tensor.matmul(
                                ps[:],
                                Wc[(l, k, m)],
                                rhs[k],
                                start=(k == 0),
                                stop=(k == nk - 1),
                            )
                        nc.vector.tensor_copy(
                            stg[:, tt * NT:(tt + 1) * NT], ps[:]
                        )
                    if l < LEVELS - 1:
                        a = actpool.tile([128, CHUNK], ADT, tag="act", name="act")
                        nc.scalar.activation(a[:], stg[:], Tanh, bias=bias[(l, m)])
                        for tt in range(TPC):
                            st["acts"][tt].append(a[:, tt * NT:(tt + 1) * NT])
                    else:
                        ob = opool.tile([128, CHUNK], F32, tag="out", name="out")
                        nc.scalar.activation(ob[:], stg[:], Tanh, bias=bias[(l, m)])
                        nc.sync.dma_start(
                            outT_d[m * 128:(m + 1) * 128,
                                   st["c0"]:st["c0"] + CHUNK],
                            ob[:],
                        )

            nchunks = BS // CHUNK

            def whole_pass():
                # two chunks in flight: emit level l of chunk A then of chunk B,
                # so the PE always has a level of matmuls to run while the other
                # chunk's tanh completes (no level-boundary stalls).
                for p in range(0, nchunks, 2):
                    stA = start_chunk(p)
                    stB = start_chunk(p + 1)
                    for l in range(LEVELS):
                        emit_level(stA, l)
                        emit_level(stB, l)

            if hw_loop:
                with tc.For_i(0, hw_loop, 1):
                    whole_pass()
            else:
                whole_pass()

    nc.compile()
    return nc


def _build_wpack(ks, bs, idxs, mode=MODE):
    """Dense per-level weights with the gather folded in, packed for SBUF."""
    wdt = np.float16 if mode == "f16" else np.float32
    wpack = np.zeros((128, NWCOLS), wdt)
    i = 0
    for l in range(LEVELS):
        C = N_IN + l * G * U
        W = np.zeros((C, G * U), np.float32)
        idx = idxs[l]
        K = ks[l]
        for g in range(G):
            np.add.at(W[:, g * U:(g + 1) * U], idx[g], K[g])
        W = W.astype(np.float16) if mode == "f16" else _round_tf32(W)
        for k in range(KCH[l]):
            for m in range(2):
                wpack[:, i * 128:(i + 1) * 128] = W[k * 128:(k + 1) * 128,
                                                    m * 128:(m + 1) * 128]
                i += 1
    bpack = np.zeros((128, 2 * LEVELS), np.float32)
    for l in range(LEVELS):
        bflat = np.asarray(bs[l], np.float32).reshape(G * U)
        for m in range(2):
            bpack[:, l * 2 + m] = bflat[m * 128:(m + 1) * 128]
    return wpack, bpack


_NC_CACHE = []


def kernel(x, k1, b1, k2, b2, k3, b3, k4, b4, idx1, idx2, idx3, idx4):
    from concourse import bass_utils

    x = np.ascontiguousarray(np.asarray(x), dtype=np.float32)
    ks = [np.asarray(a, np.float32) for a in (k1, k2, k3, k4)]
    bs = [np.asarray(a, np.float32) for a in (b1, b2, b3, b4)]
    idxs = [np.asarray(a, np.int64) for a in (idx1, idx2, idx3, idx4)]

    wpack, bpack = _build_wpack(ks, bs, idxs)

    xT = np.ascontiguousarray(x.T)  # [256, B]
    if MODE == "f16":
        xT = xT.astype(np.float16)
    else:
        xT = _round_tf32(xT)

    if not _NC_CACHE:
        _NC_CACHE.append(_build_nc())
    nc = _NC_CACHE[0]

    in_maps = [
        {"wpack": wpack, "bpack": bpack,
         "xT": np.ascontiguousarray(xT[:, c * BS:(c + 1) * BS])}
        for c in range(NCORES)
    ]
    res = bass_utils.run_bass_kernel_spmd(nc, in_maps, core_ids=list(range(NCORES)))

    out = np.empty((B, G * U), np.float32)
    for c in range(NCORES):
        out[c * BS:(c + 1) * BS, :] = res.results[c]["outT"].T
    return out


if __name__ == "__main__":
    rng = np.random.default_rng(0)
    inp = {"x": rng.standard_normal((B, N_IN), dtype=np.float32)}
    for l in range(LEVELS):
        inp[f"k{l+1}"] = (rng.standard_normal((G, F, U), dtype=np.float32) * 0.2)
        inp[f"b{l+1}"] = (rng.standard_normal((G, U), dtype=np.float32) * 0.1)
        hi = N_IN + l * (G * U)
        inp[f"idx{l+1}"] = rng.integers(0, hi, size=(G, F)).astype(np.int32)
    out = kernel(**inp)
    print("kernel out", out.shape, out.dtype, np.abs(out).max())
